# revision 2
# baseline (speedup 1.0000x reference)
"""Trainium2 Bass kernel for nn_Memcodes (vq_codebook).

Reference computation (B=4, N=1024, D=1024, H=16, C=1024, d=64):
    q = x.reshape(B,N,H,d).transpose(0,2,1,3) * d**-0.5
    k = einsum('hnd,hdc->hnc', codes, wk)
    v = einsum('hnd,hdc->hnc', codes, wv)
    logits = einsum('bhid,hjd->bhij', q, k)
    idx = argmax(logits + gumbel, -1)            # (B,H,N) int32
    out = v[h, idx]  -> (B,N,D)
    returns (out, idx)

Sharding: 8 cores = (b, g) with b in 0..4, g in 0..2; core (b,g) owns batch b,
heads g*8..g*8+8.  All inputs are laid out on the host (transposes are host-side
shard prep); all math (k/v projection, logits, argmax, gather) runs on device.

Device pipeline per (head):
    PE:   kT = wk_s^T-contract (fp32), v tiles (fp32), logits (fp32, 4cyc/row)
    DVE:  one fused custom op per n-tile: (logits+gumbel) -> running-max scan ->
          argmax index fold (single pass over PSUM+SBUF)
    ACT:  PSUM->SBUF copies for kT and v
    DMA:  gumbel stream (4MB/head), indirect row-gather of v by idx
"""
import sys

sys.path.insert(0, "/opt/trn_rl_repo")

import numpy as np

from concourse import dve_ops
from concourse.dve_spec import (
    AluOp,
    MaxNeg,
    Spec,
    Src0,
    Src1,
    Idx,
    eq,
    lower,
    maxx,
    scan,
    select,
)
from concourse.dve_uop import DveOpSpec

import concourse.bacc as bacc
import concourse.bass as bass
import concourse.mybir as mybir
from concourse.tile import TileContext
from concourse import bass_utils

# ---------------------------------------------------------------- problem dims
B, N, D, H, C = 4, 1024, 1024, 16, 1024
d = D // H  # 64
NCORES = 8
HL = H // 2  # heads per core = 8
NT = N // 128  # n-tiles = 8
SCALE = np.float32(d**-0.5)  # 1/8, exact power of two

_FLT_MAX = np.float32(3.4028235e38)

# ------------------------------------------------------------- custom DVE op


def _argmax_add_ref(in0, in1, s0, s1, imm2):
    t = in0.astype(np.float32) + in1.astype(np.float32)
    r = np.maximum.accumulate(t, axis=-1)
    idxs = np.arange(t.shape[-1], dtype=np.float32)
    rec = np.where(t == r, idxs, -_FLT_MAX).astype(np.float32)
    return rec, rec.reshape(rec.shape[0], -1).max(axis=-1, keepdims=True)


def _register_argmax_op() -> dve_ops.DveOp:
    for op in dve_ops.OPS:
        if op.name == "ARGMAX_ADD":
            return op
    t = Src0 + Src1
    r = scan(AluOp.MAX, t)
    body = select(eq(t, r), Idx, MaxNeg)
    spec = Spec(body=body, accum=maxx, accum_init=MaxNeg, reference=_argmax_add_ref)
    shas = {}
    for ver in ("v3", "v4"):
        tmp = DveOpSpec(name="ARGMAX_ADD", opcode=0, uops=lower(spec, ver=ver), rd1_en=True)
        shas[ver] = tmp.sha(ver)
    op = dve_ops.DveOp("ARGMAX_ADD", spec, subdim=False, uops_sha=shas)
    row = dve_ops._CUSTOM_DVE_ROW_BASE + len(dve_ops.OPS)
    assert row < 0x20
    dve_ops.OPS.append(op)
    dve_ops.CUSTOM_DVE_SPECS[op.name] = op.spec
    dve_ops._SUB_OPCODE_FOR_NAME[op.name] = row
    return op


ARGMAX_ADD = _register_argmax_op()

# ------------------------------------------------------------- device kernel

f32 = mybir.dt.float32
u32 = mybir.dt.uint32


def build_kernel() -> bacc.Bacc:
    nc = bacc.Bacc(trn_type="TRN2", debug=False)

    qT_d = nc.dram_tensor("qT", [HL, d, N], f32, kind="ExternalInput")
    cT_d = nc.dram_tensor("codesT", [HL, d, C], f32, kind="ExternalInput")
    wk_d = nc.dram_tensor("wk_s", [HL, d, d], f32, kind="ExternalInput")
    wv_d = nc.dram_tensor("wv", [HL, d, d], f32, kind="ExternalInput")
    gum_d = nc.dram_tensor("gum", [HL, N, C], f32, kind="ExternalInput")
    out_d = nc.dram_tensor("out_c", [N, HL * d], f32, kind="ExternalOutput")
    idx_d = nc.dram_tensor("idx_c", [HL, 128, NT], u32, kind="ExternalOutput")
    v_scr = [
        nc.dram_tensor(f"v_scr{hl}", [C, d], f32, kind="Internal") for hl in range(HL)
    ]

    out_r = out_d.ap().rearrange("(nt p) dd -> p nt dd", p=128)

    with TileContext(nc) as tc:
        with (
            tc.tile_pool(name="ld", bufs=2) as ld,       # qT/codesT/wk/wv loads
            tc.tile_pool(name="gum", bufs=2) as gp,      # gumbel stream
            tc.tile_pool(name="kv", bufs=2) as kv,       # kT / v sbuf
            tc.tile_pool(name="small", bufs=2) as sp,    # idx tiles, gather dest
            tc.tile_pool(name="scratch", bufs=1) as scr, # custom-op stream out
            tc.tile_pool(name="lgps", bufs=2, space="PSUM") as lgps,
            tc.tile_pool(name="kps", bufs=1, space="PSUM") as kps,
            tc.tile_pool(name="vps", bufs=2, space="PSUM") as vps,
        ):
            rec = scr.tile([128, C], f32)  # shared stream-out scratch
            for hl in range(HL):
                # ---- loads for this head
                qT_s = ld.tile([d, N], f32, tag="qT")
                cT_s = ld.tile([d, C], f32, tag="cT")
                wk_s = ld.tile([d, d], f32, tag="wk")
                wv_s = ld.tile([d, d], f32, tag="wv")
                nc.sync.dma_start(qT_s[:], qT_d.ap()[hl])
                nc.sync.dma_start(cT_s[:], cT_d.ap()[hl])
                nc.sync.dma_start(wk_s[:], wk_d.ap()[hl])
                nc.sync.dma_start(wv_s[:], wv_d.ap()[hl])
                gum_s = gp.tile([128, NT, C], f32, tag="gum")
                nc.sync.dma_start(
                    gum_s[:], gum_d.ap()[hl].rearrange("(nt p) c -> p nt c", p=128)
                )

                # ---- k projection: kT[j, c] = sum_d wk_s[d, j] * codesT[d, c]
                kT_ps = kps.tile([d, C], f32, tag="kT")
                nc.tensor.matmul(kT_ps[:, 0:512], lhsT=wk_s[:], rhs=cT_s[:, 0:512],
                                 start=True, stop=True)
                nc.tensor.matmul(kT_ps[:, 512:1024], lhsT=wk_s[:], rhs=cT_s[:, 512:1024],
                                 start=True, stop=True)
                kT_s = kv.tile([d, C], f32, tag="kT_s")
                nc.scalar.copy(kT_s[:], kT_ps[:])

                # ---- v: v[ct*128+p, j] = sum_d codesT[d, ct*128+p] * wv[d, j]
                v_sb = kv.tile([128, NT, d], f32, tag="v_sb")
                for ct in range(NT):
                    v_ps = vps.tile([128, d], f32, tag="v")
                    nc.tensor.matmul(
                        v_ps[:], lhsT=cT_s[:, ct * 128:(ct + 1) * 128], rhs=wv_s[:],
                        start=True, stop=True,
                    )
                    nc.scalar.copy(v_sb[:, ct, :], v_ps[:])
                nc.sync.dma_start(
                    v_scr[hl].ap().rearrange("(ct p) j -> p ct j", p=128), v_sb[:]
                )

                # ---- logits + fused argmax per n-tile
                idxf = sp.tile([128, NT], f32, tag="idxf")
                for nt in range(NT):
                    lg = lgps.tile([128, C], f32, tag="lg")
                    qpart = qT_s[:, nt * 128:(nt + 1) * 128]
                    nc.tensor.matmul(lg[:, 0:512], lhsT=qpart, rhs=kT_s[:, 0:512],
                                     start=True, stop=True)
                    nc.tensor.matmul(lg[:, 512:1024], lhsT=qpart, rhs=kT_s[:, 512:1024],
                                     start=True, stop=True)
                    nc.vector._custom_dve(
                        ARGMAX_ADD,
                        out=rec[:],
                        in0=lg[:],
                        in1=gum_s[:, nt, :],
                        accum_out=idxf[:, nt:nt + 1],
                    )

                idxu = sp.tile([128, NT], u32, tag="idxu")
                nc.vector.tensor_copy(idxu[:], idxf[:])
                nc.sync.dma_start(idx_d.ap()[hl], idxu[:])

                # ---- gather out rows: out[nt*128+p, hl*64:(hl+1)*64] = v[idx]
                for nt in range(NT):
                    gath = sp.tile([128, d], f32, tag="gath")
                    nc.gpsimd.indirect_dma_start(
                        out=gath[:],
                        out_offset=None,
                        in_=v_scr[hl].ap(),
                        in_offset=bass.IndirectOffsetOnAxis(
                            ap=idxu[:, nt:nt + 1], axis=0
                        ),
                    )
                    nc.sync.dma_start(out_r[:, nt, hl * d:(hl + 1) * d], gath[:])

    nc.finalize()
    return nc


_NC_CACHE = None


def _get_nc():
    global _NC_CACHE
    if _NC_CACHE is None:
        _NC_CACHE = build_kernel()
    return _NC_CACHE


# --------------------------------------------------------------- host wrapper


def shard_inputs(x, codes, wk, wv, gumbel):
    """Build the 8 per-core input maps (host-side layout prep only)."""
    # fold the exact 2^-3 q-scale into wk (bit-exact: power of two)
    wk_scaled = (wk * SCALE).astype(np.float32)
    codesT = np.ascontiguousarray(codes.transpose(0, 2, 1))  # (H, d, C)
    # x -> qT per (b, h): qT[d, n] = x[b, n, h*64+d]
    xT = np.ascontiguousarray(
        x.reshape(B, N, H, d).transpose(0, 2, 3, 1)
    )  # (B, H, d, N)
    in_maps = []
    for core in range(NCORES):
        b, g = divmod(core, 2)
        hs = slice(g * HL, (g + 1) * HL)
        in_maps.append(
            {
                "qT": np.ascontiguousarray(xT[b, hs]),          # (8, 64, 1024)
                "codesT": np.ascontiguousarray(codesT[hs]),     # (8, 64, 1024)
                "wk_s": np.ascontiguousarray(wk_scaled[hs]),    # (8, 64, 64)
                "wv": np.ascontiguousarray(wv[hs]),             # (8, 64, 64)
                "gum": gumbel[b, hs],                           # (8, 1024, 1024) view
            }
        )
    return in_maps


def unshard_outputs(results):
    out = np.empty((B, N, D), np.float32)
    idx = np.empty((B, H, N), np.int32)
    for core in range(NCORES):
        b, g = divmod(core, 2)
        out[b, :, g * HL * d:(g + 1) * HL * d] = results[core]["out_c"]
        # idx_c is (HL, 128, NT) with n = nt*128 + p
        idx_c = results[core]["idx_c"].astype(np.int32)  # (8, 128, 8)
        idx[b, g * HL:(g + 1) * HL] = idx_c.transpose(0, 2, 1).reshape(HL, N)
    return out, idx


def kernel(x, codes, wk, wv, gumbel):
    x = np.asarray(x, np.float32)
    codes = np.asarray(codes, np.float32)
    wk = np.asarray(wk, np.float32)
    wv = np.asarray(wv, np.float32)
    gumbel = np.asarray(gumbel, np.float32)

    nc = _get_nc()
    in_maps = shard_inputs(x, codes, wk, wv, gumbel)
    res = bass_utils.run_bass_kernel_spmd(nc, in_maps, core_ids=list(range(NCORES)))
    return unshard_outputs(res.results)


if __name__ == "__main__":
    # smoke test with random data against a numpy reference
    rng = np.random.default_rng(0)
    x = rng.standard_normal((B, N, D), dtype=np.float32)
    codes = rng.standard_normal((H, C, d), dtype=np.float32)
    wk = rng.standard_normal((H, d, d), dtype=np.float32)
    wv = rng.standard_normal((H, d, d), dtype=np.float32)
    u = rng.random((B, H, N, C), dtype=np.float32)
    gumbel = -np.log(-np.log(u + 1e-9) + 1e-9).astype(np.float32)

    out, idx = kernel(x, codes, wk, wv, gumbel)

    q = x.reshape(B, N, H, d).transpose(0, 2, 1, 3) * SCALE
    k = np.einsum("hnd,hdc->hnc", codes, wk)
    v = np.einsum("hnd,hdc->hnc", codes, wv)
    logits = np.einsum("bhid,hjd->bhij", q, k)
    idx_ref = np.argmax(logits + gumbel, axis=-1)
    out_ref = v[np.arange(H)[None, :, None], idx_ref].transpose(0, 2, 1, 3).reshape(B, N, D)
    print("idx match frac:", (idx == idx_ref).mean())
    err = np.linalg.norm(out - out_ref) / np.linalg.norm(out_ref)
    print("out rel err:", err)
